# revision 6
# baseline (speedup 1.0000x reference)
"""Trainium2 Bass kernel v2 for nn_AutoMemoryModule (scatter_memory).

Two SPMD launches over 8 NeuronCores, host reduction in between:

Launch 1 (mm2): K-sharded first-layer matmul, fp16 hi/lo pairs.
  w1 is split w = hi + lo/2^13 with both parts fp16 (exact to ~2^-22 rel,
  verified to give logit error ~6e-9 on this data — 20x under the smallest
  adjacent top-k gap of 1.2e-7). Each core streams its 8 MB slice as 32
  groups of [128, 2, 512] and runs 64 matmuls [128K,32M]x[128K,512N]
  (1 cyc/col fp16 -> ~14 us tensor, hidden under ~27 us DMA; the fp32
  baseline was tensor-bound at 4 cyc/col, 31 us). The x operand (gathered
  embeddings, also hi/lo split) is packed per group into the stationary
  side with (chunk, stream, split) along M; block-diagonal entries of the
  two [32,512] PSUM accumulators hold the partial pre-activations. Host
  recombines diagonals with the 2^-13 scales and reduces across cores.

Launch 2 (tail2): scatter/dedup/rank, distributed 64 candidates/core.
  Every core computes all 512 logits from the reduced h (two small fp32
  matmuls against w2 whose bias row also carries (valid-1)*1e20, masking
  pad candidates for free). Duplicate-token pairs sit at host-permuted
  fixed slots, so group-max/dedup is a handful of strided max/min ops
  (winner keeps its score, loser drops to ~-1e20). The masked score row
  is broadcast to a [64,512] PSUM tile via a ones-column matmul; each
  core selects its own candidates' scores with a one-hot row-dot and
  computes rank = #(c_p > c_q). The tie term is omitted on device:
  exact-fp32-tie collisions among kept candidates are detected by the
  host and resolved token-ascending (identical to top_k semantics), and
  suppressed candidates all collapse to exactly -1e20 and are dropped.
  Output is (masked score, rank) per candidate; the host scatters by
  rank into the 256 output slots and applies the f64 sigmoid.

Sync discipline: walrus allows one semaphore wait per instruction;
_split_multi_waits hoists extra waits onto same-engine NOPs.
"""
import sys
import numpy as np

sys.path.insert(0, "/opt/trn_rl_repo")

import concourse.bass as bass
import concourse.tile as tile
from concourse import mybir
from concourse.bass_utils import run_bass_kernel_spmd

F32 = mybir.dt.float32
F16 = mybir.dt.float16
NEG = np.float32(-1e20)
BIG = 1.0e20
VOCAB, MSL, EMB = 32000, 256, 1024
NCORES = 8
KTOT = EMB * MSL            # 262144
KSH = KTOT // NCORES        # 32768 per core
NGRP = 32                   # groups of 8 chunks per core
SC = np.float32(2.0 ** 13)  # lo-part scale

Alu = mybir.AluOpType


def _split_multi_waits(nc):
    """Hoist all but one semaphore wait of each instruction onto same-engine
    NOPs (walrus rejects multi-wait instructions)."""
    import copy
    templates = {}
    for fn in nc.m.functions:
        for bb in fn.blocks:
            for ins in bb.instructions:
                if type(ins).__name__ == "InstEventSemaphore" \
                        and ins.engine not in templates:
                    templates[ins.engine] = ins
    n = [0]

    def make_nop(eng, w):
        nop = copy.deepcopy(templates[eng])
        n[0] += 1
        nop.name = f"WS-{n[0]}"
        nop.sync_info = mybir.SyncInfo(on_wait=[w], on_update=[])
        return nop

    for fn in nc.m.functions:
        for bb in fn.blocks:
            out = []
            for ins in bb.instructions:
                si = getattr(ins, "sync_info", None)
                if si is not None and si.on_wait and len(si.on_wait) > 1:
                    waits = list(si.on_wait)
                    for w in waits[:-1]:
                        out.append(make_nop(ins.engine, w))
                    si.on_wait = [waits[-1]]
                out.append(ins)
            bb.instructions[:] = out


def build_mm2(gper=2):
    """Launch 1: fp16 hi/lo grouped matmul. gper = groups per DMA descriptor
    (NGRP/gper descriptors alternating the two HWDGE trigger engines)."""
    nc = bass.Bass()
    ph_d = nc.dram_tensor("pho", [32, 512], F32, kind="ExternalOutput")
    pl_d = nc.dram_tensor("plo", [32, 512], F32, kind="ExternalOutput")
    # host layout: wg[g, p, t, n]  (group, K-partition, hi/lo, col=c*64+j)
    wg_d = nc.dram_tensor("wg", [NGRP, 128, 2, 512], F16, kind="ExternalInput")
    xg_d = nc.dram_tensor("xg", [128, NGRP, 32], F16, kind="ExternalInput")
    with tile.TileContext(nc) as tc:
        with tc.tile_pool(name="pool", bufs=1) as pool, \
             tc.tile_pool(name="psum", bufs=1, space="PSUM") as psum:
            xt = pool.tile([128, NGRP, 32], F16)
            nc.scalar.dma_start(xt[:], xg_d[:])
            wts = []
            nd = NGRP // gper
            for d in range(nd):
                wt = pool.tile([128, gper, 2, 512], F16, tag=f"wt{d}")
                src = wg_d[d * gper:(d + 1) * gper]          # [gper,128,2,512]
                (nc.sync if d % 2 == 0 else nc.scalar).dma_start(
                    wt[:], src.rearrange("g p t n -> p g t n"))
                wts.append(wt)
            psh = psum.tile([32, 512], F32)
            psl = psum.tile([32, 512], F32)
            for g in range(NGRP):
                wt = wts[g // gper]
                gg = g % gper
                nc.tensor.matmul(psh[:], xt[:, g, :], wt[:, gg, 0, :],
                                 start=(g == 0), stop=(g == NGRP - 1),
                                 skip_group_check=True)
                nc.tensor.matmul(psl[:], xt[:, g, :], wt[:, gg, 1, :],
                                 start=(g == 0), stop=(g == NGRP - 1),
                                 skip_group_check=True)
            oh = pool.tile([32, 512], F32)
            nc.vector.tensor_copy(oh[:], psh[:])
            ol = pool.tile([32, 512], F32)
            nc.vector.tensor_copy(ol[:], psl[:])
            nc.sync.dma_start(ph_d[:], oh[:])
            nc.scalar.dma_start(pl_d[:], ol[:])
    _split_multi_waits(nc)
    return nc




def build_mm2_pmajor(gper=2):
    """Launch 1 variant: p-major weight layout [128, NGRP, 2, 512] so each
    DMA descriptor moves 4KB-contiguous per-partition lines. PSUM halves are
    copied/DMA'd as they finish so the out path overlaps the stream."""
    nc = bass.Bass()
    ph_d = nc.dram_tensor("pho", [2, 32, 512], F32, kind="ExternalOutput")
    pl_d = nc.dram_tensor("plo", [2, 32, 512], F32, kind="ExternalOutput")
    wg_d = nc.dram_tensor("wgp", [128, NGRP, 2, 512], F16, kind="ExternalInput")
    xg_d = nc.dram_tensor("xg", [128, NGRP, 32], F16, kind="ExternalInput")
    with tile.TileContext(nc) as tc:
        with tc.tile_pool(name="pool", bufs=1) as pool, \
             tc.tile_pool(name="psum", bufs=1, space="PSUM") as psum:
            # x for the first half rides ahead of the weight stream; the
            # second half lands behind two weight descriptors, well before
            # group NGRP/2 needs it
            xta = pool.tile([128, NGRP // 2, 32], F16)
            nc.scalar.dma_start(xta[:], xg_d[:, 0:NGRP // 2])
            xtb = pool.tile([128, NGRP // 2, 32], F16)
            wts = []
            nd = NGRP // gper
            for d in range(nd):
                wt = pool.tile([128, gper, 2, 512], F16, tag=f"wt{d}")
                (nc.sync if d % 2 == 0 else nc.scalar).dma_start(
                    wt[:], wg_d[:, d * gper:(d + 1) * gper])
                wts.append(wt)
                if d == 3:
                    nc.scalar.dma_start(xtb[:], xg_d[:, NGRP // 2:])
            half = NGRP // 2
            ps = {(t, hf): psum.tile([32, 512], F32, name=f"ps{t}{hf}",
                                      tag=f"ps{t}{hf}")
                  for t in range(2) for hf in range(2)}
            oh = pool.tile([32, 512], F32)
            ol = pool.tile([32, 512], F32)
            for g in range(NGRP):
                wt = wts[g // gper]
                gg = g % gper
                hf = g // half
                xs = xta[:, g, :] if g < NGRP // 2 else xtb[:, g - NGRP // 2, :]
                for t in range(2):
                    nc.tensor.matmul(ps[(t, hf)][:], xs,
                                     wt[:, gg, t, :],
                                     start=(g % half == 0),
                                     stop=(g % half == half - 1),
                                     skip_group_check=True)
                if g == half:
                    # first-half copies + DMA overlap the stream's second half
                    nc.vector.tensor_copy(oh[:], ps[(0, 0)][:])
                    nc.vector.tensor_copy(ol[:], ps[(1, 0)][:])
                    nc.sync.dma_start(ph_d[0], oh[:])
                    nc.scalar.dma_start(pl_d[0], ol[:])
            oh2 = pool.tile([32, 512], F32)
            nc.vector.tensor_copy(oh2[:], ps[(0, 1)][:])
            ol2 = pool.tile([32, 512], F32)
            nc.vector.tensor_copy(ol2[:], ps[(1, 1)][:])
            nc.sync.dma_start(ph_d[1], oh2[:])
            nc.scalar.dma_start(pl_d[1], ol2[:])
    _split_multi_waits(nc)
    return nc

def build_tail2(n00, n11, n01, warmup=True):
    """Launch 2: distributed tail. Dup pairs are host-permuted to fixed slots:
    class00 at (2j, 2j+1), class11 at (256+2j, 256+2j+1), class01 at
    (128+j, 384+j); counts are baked into the instruction stream."""
    nc = bass.Bass()
    out_d = nc.dram_tensor("outv", [64, 2], F32, kind="ExternalOutput")
    # w2b: [65, 512] block-diag w2 with per-stream slot-permuted columns,
    # row 64 = b2 + (valid-1)*BIG (bias + invalid mask ride the matmul via
    # the ones row of hcol2)
    w2b_d = nc.dram_tensor("w2b", [65, 512], F32, kind="ExternalInput")
    h_d = nc.dram_tensor("hcol2", [65, 2], F32, kind="ExternalInput")
    # selq: one-hot own-block selection, folded layout [q + 64h, p'] over
    # p-halves h so DVE ops use all 128 lanes
    sel_d = nc.dram_tensor("selq", [128, 256], F32, kind="ExternalInput")
    with tile.TileContext(nc) as tc:
        with tc.tile_pool(name="pool", bufs=1) as pool, \
             tc.tile_pool(name="psum", bufs=1, space="PSUM") as psum:
            w2b = pool.tile([65, 512], F32)
            nc.sync.dma_start(w2b[:], w2b_d[:])
            hc = pool.tile([65, 2], F32)
            nc.sync.dma_start(hc[:], h_d[:])
            selq = pool.tile([128, 256], F32)
            nc.scalar.dma_start(selq[:], sel_d[:])

            ones1 = pool.tile([1, 64], F32)
            nc.vector.memset(ones1[:], 1.0)
            if warmup:
                # small PE warmup: lifts the clock out of the cold p-state
                # before the real matmuls (N=64 keeps the warmup cheap)
                wrow = pool.tile([1, 64], F32)
                nc.vector.memset(wrow[:], 0.0)
                warm = psum.tile([64, 64], F32)
                nc.tensor.matmul(warm[:], ones1[:], wrow[:], start=True,
                                 stop=True)
                nc.tensor.matmul(warm[:], ones1[:], wrow[:], start=True,
                                 stop=True)

            # layer 2: z = hr @ [w2;b2]  (hr = relu'd h + ones row, from the
            # host-side reduction; both streams, block-diagonal)
            zps = psum.tile([1, 512], F32)
            nc.tensor.matmul(zps[0:1, 0:256], hc[:, 0:1], w2b[:, 0:256],
                             start=True, stop=True)
            nc.tensor.matmul(zps[0:1, 256:512], hc[:, 1:2], w2b[:, 256:512],
                             start=True, stop=True)

            # dup group-max + dedup on permuted fixed slots: the winner
            # (max) keeps its score, the loser (min) is pushed to ~-1e20;
            # exact ties give both the same score and the same token, so
            # which slot survives is irrelevant. Invalid slots were already
            # masked by the bias row.
            cm = pool.tile([1, 512], F32)
            nc.vector.tensor_copy(cm[:], zps[:])
            spans = []
            for base, n in ((0, n00), (256, n11)):
                if n:
                    spans.append((slice(base, base + 2 * n, 2),
                                  slice(base + 1, base + 2 * n + 1, 2)))
            if n01:
                spans.append((slice(128, 128 + n01), slice(384, 384 + n01)))
            tmx = pool.tile([1, 64], F32)
            for si, (e, o) in enumerate(spans):
                n = len(range(*e.indices(512)))
                nc.vector.tensor_tensor(tmx[0:1, 0:n], cm[0:1, e], cm[0:1, o],
                                        Alu.max)
                nc.vector.tensor_tensor(tmx[0:1, 32:32 + n], cm[0:1, e],
                                        cm[0:1, o], Alu.min)
                nc.vector.tensor_copy(cm[0:1, e], tmx[0:1, 0:n])
                nc.vector.tensor_scalar(cm[0:1, o], tmx[0:1, 32:32 + n], 1.0,
                                        -BIG, Alu.mult, Alu.add)

            # broadcast cm folded to [128, 256]: rows 0:64 hold candidates
            # vs p-half 0, rows 64:128 vs p-half 1 (full DVE lane use)
            cmB = psum.tile([128, 256], F32)
            nc.tensor.matmul(cmB[0:64, :], ones1[:], cm[0:1, 0:256],
                             start=True, stop=True)
            nc.tensor.matmul(cmB[64:128, :], ones1[:], cm[0:1, 256:512],
                             start=True, stop=True)

            ones2 = pool.tile([128, 256], F32)
            nc.gpsimd.memset(ones2[:], 1.0)

            # cq = own-block scores via fused one-hot row-dot over both halves
            J = pool.tile([128, 256], F32)
            cqh = pool.tile([128, 1], F32)
            nc.vector.scalar_tensor_tensor(J[:], cmB[:], 1.0, selq[:],
                                           Alu.mult, Alu.mult,
                                           accum_out=cqh[:])
            cqt = pool.tile([64, 1], F32)
            nc.vector.tensor_copy(cqt[:], cqh[64:128])
            cq = pool.tile([64, 1], F32)
            nc.vector.tensor_tensor(cq[:], cqh[0:64], cqt[:], Alu.add)
            cq128 = pool.tile([128, 1], F32)
            nc.vector.tensor_copy(cq128[0:64], cq[:])
            nc.vector.tensor_copy(cq128[64:128], cq[:])

            # rank = sum(cmB > cq) across both halves; exact-tie collisions
            # among kept candidates are resolved token-ascending by the host
            RJ = pool.tile([128, 256], F32)
            rankh = pool.tile([128, 1], F32)
            nc.vector.scalar_tensor_tensor(RJ[:], cmB[:], cq128[:], ones2[:],
                                           Alu.is_gt, Alu.mult,
                                           accum_out=rankh[:])
            rkt = pool.tile([64, 1], F32)
            nc.vector.tensor_copy(rkt[:], rankh[64:128])
            rank = pool.tile([64, 1], F32)
            nc.vector.tensor_tensor(rank[:], rankh[0:64], rkt[:], Alu.add)

            outv = pool.tile([64, 2], F32)
            nc.vector.tensor_copy(outv[:, 0:1], cq[:])
            nc.vector.tensor_copy(outv[:, 1:2], rank[:])
            nc.sync.dma_start(out_d[:], outv[:])
    _split_multi_waits(nc)
    return nc


_cache = {}


def _get_mm2():
    if "mm2" not in _cache:
        _cache["mm2"] = build_mm2_pmajor()
    return _cache["mm2"]


def _get_tail2(n00, n11, n01):
    key = ("tail2", n00, n11, n01)
    if key not in _cache:
        _cache[key] = build_tail2(n00, n11, n01)
    return _cache[key]


def _split16(a):
    hi = a.astype(np.float16)
    lo = ((a - hi.astype(np.float32)) * SC).astype(np.float16)
    return hi, lo


def kernel(input_tokens, memory_context, emb_table, w1, b1, w2, b2,
           _trace=False, _tmpdir=None):
    it = np.asarray(input_tokens).astype(np.int64)
    mc = np.asarray(memory_context).astype(np.int64)
    emb = np.asarray(emb_table, dtype=np.float32)
    w1 = np.asarray(w1, dtype=np.float32)
    b1 = np.asarray(b1, dtype=np.float32)
    w2 = np.asarray(w2, dtype=np.float32)
    b2 = np.asarray(b2, dtype=np.float32)

    padded = np.zeros(MSL, np.int64)
    padded[:it.shape[0]] = it
    comb = np.concatenate([padded, mc])

    # ---- launch 1 host prep ----
    x = np.stack([emb[padded].reshape(-1), emb[mc].reshape(-1)])  # [2, KTOT]
    x_hi, x_lo = _split16(x)
    w_hi, w_lo = _split16(w1)                                     # [KTOT, 64]
    per_core = []
    for i in range(NCORES):
        k0 = KSH * i
        wh = w_hi[k0:k0 + KSH].reshape(NGRP, 8, 128, 64)   # g c p j
        wl = w_lo[k0:k0 + KSH].reshape(NGRP, 8, 128, 64)
        wtc = np.stack([wh, wl], axis=0)                   # t g c p j
        wgp = np.ascontiguousarray(
            wtc.transpose(3, 1, 0, 2, 4).reshape(128, NGRP, 2, 512))
        xs = np.stack([x_hi[:, k0:k0 + KSH], x_lo[:, k0:k0 + KSH]])  # t s K
        xs = xs.reshape(2, 2, NGRP, 8, 128)                # t s g c p
        xg = np.ascontiguousarray(
            xs.transpose(4, 2, 3, 1, 0).reshape(128, NGRP, 32))
        per_core.append({"wgp": wgp, "xg": xg})

    nc1 = _get_mm2()
    res1 = run_bass_kernel_spmd(nc1, per_core, core_ids=list(range(NCORES)),
                                trace=_trace, tmpdir=_tmpdir)

    # host recombine: diagonal blocks with scales, reduce across cores
    h = np.zeros((2, 64), np.float64)
    scl = float(SC)
    for r in res1.results:
        ph4 = r["pho"].astype(np.float64).sum(0).reshape(8, 2, 2, 8, 64)
        pl4 = r["plo"].astype(np.float64).sum(0).reshape(8, 2, 2, 8, 64)
        for cc in range(8):
            for t in range(2):
                h[0] += ph4[cc, 0, t, cc] / scl ** t + pl4[cc, 0, t, cc] / scl ** (t + 1)
                h[1] += ph4[cc, 1, t, cc] / scl ** t + pl4[cc, 1, t, cc] / scl ** (t + 1)
    h = h.astype(np.float32)

    # ---- launch 2 host prep ----
    valid = comb != 0
    uniq = np.unique(comb)
    ordv = np.searchsorted(uniq, comb).astype(np.float32)
    dup_pairs = []
    seen = {}
    for q in range(512):
        if valid[q]:
            t = int(comb[q])
            if t in seen:
                dup_pairs.append((seen[t], q))
            else:
                seen[t] = q

    # permute candidates to fixed dup-slots: class00 (2j,2j+1),
    # class11 (256+2j,..), class01 (128+j, 384+j)
    p00 = [p for p in dup_pairs if p[0] < 256 and p[1] < 256]
    p11 = [p for p in dup_pairs if p[0] >= 256 and p[1] >= 256]
    p01 = [p for p in dup_pairs if p[0] < 256 <= p[1]]
    n00, n11, n01 = len(p00), len(p11), len(p01)
    assert 2 * n00 <= 128 and 2 * n11 <= 128 and n01 <= 128
    perm = np.full(512, -1, np.int64)       # perm[slot] = original q
    for j, (a, b) in enumerate(p00):
        perm[2 * j], perm[2 * j + 1] = a, b
    for j, (a, b) in enumerate(p11):
        perm[256 + 2 * j], perm[256 + 2 * j + 1] = a, b
    for j, (a, b) in enumerate(p01):
        perm[128 + j], perm[384 + j] = a, b
    used = set(perm[perm >= 0].tolist())
    free0 = [q for q in range(256) if q not in used]
    free1 = [q for q in range(256, 512) if q not in used]
    for u in range(256):
        if perm[u] < 0:
            perm[u] = free0.pop(0)
    for u in range(256, 512):
        if perm[u] < 0:
            perm[u] = free1.pop(0)
    assert not free0 and not free1

    hcol2 = np.ones((65, 2), np.float32)
    hcol2[0:64] = np.maximum(h + b1[None, :], 0.0).T
    w2b = np.zeros((65, 512), np.float32)
    b2d = np.concatenate([b2, b2])
    for u in range(512):
        w2b[0:64, u] = w2[:, perm[u] % 256]
        w2b[64, u] = b2d[perm[u]] + (float(valid[perm[u]]) - 1.0) * BIG
    in_maps = []
    for i in range(NCORES):
        selq = np.zeros((128, 256), np.float32)
        for q in range(64):
            hh, pp = divmod(64 * i + q, 256)
            selq[q + 64 * hh, pp] = 1.0
        in_maps.append({"w2b": w2b, "hcol2": hcol2, "selq": selq})

    nc2 = _get_tail2(n00, n11, n01)
    res2 = run_bass_kernel_spmd(nc2, in_maps, core_ids=list(range(NCORES)),
                                trace=_trace)

    vals = np.concatenate([res2.results[i]["outv"][:, 0] for i in range(NCORES)])
    ranks = np.rint(np.concatenate(
        [res2.results[i]["outv"][:, 1] for i in range(NCORES)])).astype(np.int64)
    toks = comb[perm]
    keep = vals > -5e19
    # device rank omits the tie term; bitwise-equal kept scores collide on the
    # same rank — resolve each collision group token-ascending (top_k order)
    tok_out = np.zeros(256, np.int32)
    sc_out = np.full(256, NEG, np.float32)
    by_rank = {}
    for u in np.nonzero(keep)[0]:
        by_rank.setdefault(int(ranks[u]), []).append(u)
    for r0, group in by_rank.items():
        group.sort(key=lambda u: toks[u])
        for off, u in enumerate(group):
            r = r0 + off
            if r < 256:
                tok_out[r] = np.int32(toks[u])
                sc_out[r] = np.float32(1.0 / (1.0 + np.exp(-np.float64(vals[u]))))
    kernel.last_result = (res1, res2)
    return tok_out, sc_out
